# revision 1
# baseline (speedup 1.0000x reference)
"""Trainium2 Bass kernel for nn_Aligner: softmax-over-tokens alignment.

    out[b, d] = sum_a sum_t hidden[b, d, t] * softmax_t(-(c[b,t]-ts[b,a])^2/100)[a, t]

Algorithm (per batch):
  exp(-(d/10)^2) underflows to exactly 0 in f32 for |d| > ~102, so for each
  chunk of audio frames only a narrow window of token centers matters.
  Host-side planning (index arithmetic only) picks, for each group of frame
  chunks, a center window [t_start, t_start+W). The device computes the
  banded Gaussians, row-normalizes (softmax), reduces over frames with
  TensorE mask matmuls into window-local weight sums uwT[w, group], and
  contracts against a window-local gather of hidden (host gather, bf16) with
  16 accumulating matmuls. Data-parallel over batch: 2 batches per core on 8
  NeuronCores. All access patterns are value-independent (SPMD-safe); data
  dependence lives only in the host-prepared input arrays.
"""
import numpy as np
import ml_dtypes
import concourse.bacc as bacc
import concourse.mybir as mybir
from concourse import tile
from concourse.bass_utils import run_bass_kernel_spmd

B, T, A, D = 16, 512, 4096, 256
TEMP = 10.0
R_BAND = 102.0          # |d| band: exp(-(102^2)/100) == 0 in f32 (denormal floor)
C = 8                   # frames per partition chunk
G = 32                  # partitions per group (shared center window)
N_CORES = 8
BPC = B // N_CORES      # batches per core
N_TILES = A // (128 * C)        # 4 a-tiles per batch
N_GROUPS = 128 // G             # 4 groups per tile
N_GID = N_TILES * N_GROUPS      # 16 windows per batch

dt = mybir.dt

_build_cache = {}


def _plan_windows(centers_b, ts_b):
    """t_starts[n_tiles, n_groups], needed W for one batch. Requires sorted
    centers (reference sorts them); caller falls back to dense if not."""
    t_starts = np.zeros((N_TILES, N_GROUPS), dtype=np.int64)
    t_ends = np.zeros((N_TILES, N_GROUPS), dtype=np.int64)
    for tau in range(N_TILES):
        for g in range(N_GROUPS):
            f_lo = tau * 128 * C + g * G * C
            f_hi = f_lo + G * C
            seg = ts_b[f_lo:f_hi]
            t_starts[tau, g] = np.searchsorted(centers_b, seg.min() - R_BAND, "left")
            t_ends[tau, g] = np.searchsorted(centers_b, seg.max() + R_BAND, "right")
    return t_starts, int((t_ends - t_starts).max())


def _build(W):
    """Build + compile the SPMD program (identical on all 8 cores)."""
    if W in _build_cache:
        return _build_cache[W]

    nc = bacc.Bacc("TRN2", target_bir_lowering=False, debug=False,
                   num_devices=N_CORES)
    cc_d = nc.dram_tensor("cc2", [BPC, N_TILES, 128, W], dt.float32,
                          kind="ExternalInput")
    tsd_d = nc.dram_tensor("tsd", [BPC, N_TILES, 128, C], dt.float32,
                           kind="ExternalInput")
    hgt_d = nc.dram_tensor("hgt", [BPC, N_GID, W, D], dt.bfloat16,
                           kind="ExternalInput")
    out_d = nc.dram_tensor("out", [BPC, D], dt.float32, kind="ExternalOutput")

    with tile.TileContext(nc) as tc:
        with tc.tile_pool(name="pool", bufs=2) as pool, \
             tc.tile_pool(name="hgp", bufs=3) as hgp, \
             tc.tile_pool(name="psum", bufs=1, space="PSUM") as psum_pool, \
             tc.tile_pool(name="const", bufs=1) as cpool, \
             nc.allow_low_precision(reason="softmax weights tolerate bf16"):

            # mask[p, g] = 1 if partition p belongs to group g
            mask_t = cpool.tile([128, N_GROUPS], dt.bfloat16)
            nc.vector.memset(mask_t[:], 0.0)
            for g in range(N_GROUPS):
                nc.vector.memset(mask_t[g * G:(g + 1) * G, g:g + 1], 1.0)

            for slot in range(BPC):
                psum_uwT = psum_pool.tile([W, N_GID], dt.float32,
                                          tag=f"uwT{slot}")
                for tau in range(N_TILES):
                    cc_t = pool.tile([128, W], dt.float32, tag="cc")
                    nc.sync.dma_start(out=cc_t[:], in_=cc_d.ap()[slot, tau])
                    tsd_t = pool.tile([128, C], dt.float32, tag="tsd")
                    nc.sync.dma_start(out=tsd_t[:], in_=tsd_d.ap()[slot, tau])

                    # d[p, w, i] = cc[p, w] - tsd[p, i]
                    d_t = pool.tile([128, W, C], dt.bfloat16, tag="d")
                    nc.vector.tensor_tensor(
                        out=d_t[:],
                        in0=cc_t[:].unsqueeze(2).broadcast_to([128, W, C]),
                        in1=tsd_t[:].unsqueeze(1).broadcast_to([128, W, C]),
                        op=mybir.AluOpType.subtract)

                    sq_t = pool.tile([128, W, C], dt.bfloat16, tag="sq")
                    nc.scalar.square(out=sq_t[:], in_=d_t[:])

                    # e = exp(-sq / TEMP^2)
                    e_t = pool.tile([128, W, C], dt.bfloat16, tag="e")
                    nc.scalar.activation(out=e_t[:], in_=sq_t[:],
                                         func=mybir.ActivationFunctionType.Exp,
                                         scale=-(TEMP ** -2))

                    # softmax row sums over the window: s[p, i] = sum_w e
                    s_t = pool.tile([128, C], dt.float32, tag="s")
                    nc.vector.reduce_sum(out=s_t[:],
                                         in_=e_t[:].transpose([0, 2, 1]),
                                         axis=mybir.AxisListType.X)
                    r_t = pool.tile([128, C], dt.bfloat16, tag="r")
                    nc.vector.reciprocal(out=r_t[:], in_=s_t[:])

                    # normalized weights m = e * r
                    m_t = pool.tile([128, W, C], dt.bfloat16, tag="m")
                    nc.vector.tensor_tensor(
                        out=m_t[:], in0=e_t[:],
                        in1=r_t[:].unsqueeze(1).broadcast_to([128, W, C]),
                        op=mybir.AluOpType.mult)

                    # u[p, w] = sum_i m[p, w, i]
                    u_t = pool.tile([128, W], dt.bfloat16, tag="u")
                    nc.vector.reduce_sum(out=u_t[:], in_=m_t[:],
                                         axis=mybir.AxisListType.X)

                    # uwT[w, groups of tau] += u.T @ mask
                    nc.tensor.matmul(
                        out=psum_uwT[:, tau * N_GROUPS:(tau + 1) * N_GROUPS],
                        lhsT=u_t[:], rhs=mask_t[:],
                        start=(tau == 0), stop=(tau == N_TILES - 1))

                uwT_sb = pool.tile([W, N_GID], dt.bfloat16, tag="uwTs")
                nc.scalar.copy(out=uwT_sb[:], in_=psum_uwT[:])

                # out[d] = sum_gid uwT[:, gid].T @ hgt[gid]
                psum_out = psum_pool.tile([1, D], dt.float32, tag=f"out{slot}")
                for gid in range(N_GID):
                    hg_t = hgp.tile([W, D], dt.bfloat16, tag="hg")
                    nc.sync.dma_start(out=hg_t[:], in_=hgt_d.ap()[slot, gid])
                    nc.tensor.matmul(out=psum_out[:],
                                     lhsT=uwT_sb[:, gid:gid + 1], rhs=hg_t[:],
                                     start=(gid == 0), stop=(gid == N_GID - 1))

                out_sb = pool.tile([1, D], dt.float32, tag="osb")
                nc.scalar.copy(out=out_sb[:], in_=psum_out[:])
                nc.sync.dma_start(out=out_d.ap()[slot].unsqueeze(0),
                                  in_=out_sb[:])

    nc.compile()
    _build_cache[W] = nc
    return nc


def _prepare(hidden_state, centers, audio_timestamps):
    """Host planning + gathers. Returns (W, in_maps)."""
    hidden_state = np.ascontiguousarray(hidden_state, dtype=np.float32)
    centers = np.ascontiguousarray(centers, dtype=np.float32)
    ts = np.ascontiguousarray(audio_timestamps, dtype=np.float32)

    sorted_ok = all(np.all(np.diff(centers[b]) >= 0) for b in range(B))

    all_starts = np.zeros((B, N_TILES, N_GROUPS), dtype=np.int64)
    W = 0
    if sorted_ok:
        for b in range(B):
            st, w = _plan_windows(centers[b], ts[b])
            all_starts[b] = st
            W = max(W, w)
        W = min((W + 7) // 8 * 8, T)
    if not sorted_ok or W >= T:
        W = T
        all_starts[:] = 0
    all_starts = np.clip(np.minimum(all_starts, T - W), 0, None)

    # gathers (vectorized): windows[b, tau, g] -> slice of centers / hidden
    idx = all_starts[..., None] + np.arange(W)          # [B, nt, ng, W]
    cc2 = np.empty((B, N_TILES, 128, W), dtype=np.float32)
    tsd = np.empty((B, N_TILES, 128, C), dtype=np.float32)
    hgt = np.empty((B, N_GID, W, D), dtype=ml_dtypes.bfloat16)
    for b in range(B):
        cwin = centers[b][idx[b]]                       # [nt, ng, W]
        # per-partition: group g of tile tau covers partitions [gG, (g+1)G)
        cc2[b] = np.repeat(cwin, G, axis=1)             # [nt, 128, W]
        tsb = ts[b].reshape(N_TILES, 128, C)
        base = tsb[:, :, 0:1]
        cc2[b] -= base
        tsd[b] = tsb - base
        # hgt[gid, w, d] = hidden[b, d, t_start+w]
        hg = hidden_state[b][:, idx[b].reshape(N_GID, W)]   # [D, ngid, W]
        hgt[b] = hg.transpose(1, 2, 0).astype(ml_dtypes.bfloat16)

    in_maps = []
    for k in range(N_CORES):
        bs = slice(k * BPC, (k + 1) * BPC)
        in_maps.append({
            "cc2": np.ascontiguousarray(cc2[bs]),
            "tsd": np.ascontiguousarray(tsd[bs]),
            "hgt": np.ascontiguousarray(hgt[bs]),
        })
    return W, in_maps


def kernel(hidden_state, centers, audio_timestamps):
    W, in_maps = _prepare(hidden_state, centers, audio_timestamps)
    nc = _build(W)
    res = run_bass_kernel_spmd(nc, in_maps, core_ids=list(range(N_CORES)))
    out = np.empty((B, D), dtype=np.float32)
    for k in range(N_CORES):
        out[k * BPC:(k + 1) * BPC] = res.results[k]["out"]
    return out


# revision 4
# speedup vs baseline: 23.0271x; 23.0271x over previous
"""Trainium2 Bass kernel for nn_Aligner: softmax-over-tokens alignment.

    out[b, d] = sum_a sum_t hidden[b, d, t] * softmax_t(-(c[b,t]-ts[b,a])^2/100)[a, t]

Algorithm (per batch):
  exp(-(d/10)^2) underflows to exactly 0 in f32 for |d| > ~102, so for each
  chunk of audio frames only a narrow window of token centers matters.
  Host-side planning (index arithmetic only) picks, for each group of frame
  chunks, a center window [t_start, t_start+W). The device computes the
  banded Gaussians, row-normalizes (softmax), reduces over frames with
  TensorE mask matmuls into window-local weight sums uwT[w, group], and
  contracts against a window-local gather of hidden (host gather, bf16) with
  16 accumulating matmuls. Data-parallel over batch: 2 batches per core on 8
  NeuronCores. All access patterns are value-independent (SPMD-safe); data
  dependence lives only in the host-prepared input arrays.
"""
import numpy as np
import ml_dtypes
import concourse.bacc as bacc
import concourse.mybir as mybir
from concourse import tile
from concourse.bass_utils import run_bass_kernel_spmd

B, T, A, D = 16, 512, 4096, 256
TEMP = 10.0
R_BAND = 102.0          # |d| band: exp(-(102^2)/100) == 0 in f32 (denormal floor)
C = 8                   # frames per partition chunk
G = 32                  # partitions per group (shared center window)
N_CORES = 8
BPC = B // N_CORES      # batches per core
N_TILES = A // (128 * C)        # 4 a-tiles per batch
N_GROUPS = 128 // G             # 4 groups per tile
N_GID = N_TILES * N_GROUPS      # 16 windows per batch

dt = mybir.dt

_build_cache = {}


def _plan_windows(centers_b, ts_b):
    """t_starts[n_tiles, n_groups], needed W for one batch. Requires sorted
    centers (reference sorts them); caller falls back to dense if not."""
    t_starts = np.zeros((N_TILES, N_GROUPS), dtype=np.int64)
    t_ends = np.zeros((N_TILES, N_GROUPS), dtype=np.int64)
    for tau in range(N_TILES):
        for g in range(N_GROUPS):
            f_lo = tau * 128 * C + g * G * C
            f_hi = f_lo + G * C
            seg = ts_b[f_lo:f_hi]
            t_starts[tau, g] = np.searchsorted(centers_b, seg.min() - R_BAND, "left")
            t_ends[tau, g] = np.searchsorted(centers_b, seg.max() + R_BAND, "right")
    return t_starts, int((t_ends - t_starts).max())


def _build(W, reps=1):
    """Build + compile the SPMD program (identical on all 8 cores).

    reps > 1 repeats the whole computation inside one NEFF — used by the
    test harness to amortize dispatch overhead when timing on hardware.
    """
    if (W, reps) in _build_cache:
        return _build_cache[(W, reps)]

    nc = bacc.Bacc("TRN2", target_bir_lowering=False, debug=False,
                   num_devices=N_CORES)
    cc_d = nc.dram_tensor("cc2", [BPC, N_TILES, 128, W], dt.float32,
                          kind="ExternalInput")
    tsd_d = nc.dram_tensor("tsd", [BPC, N_TILES, 128, C], dt.float32,
                           kind="ExternalInput")
    hgt_d = nc.dram_tensor("hgt", [BPC, N_GID, W, D], dt.bfloat16,
                           kind="ExternalInput")
    out_d = nc.dram_tensor("out", [BPC, D], dt.float32, kind="ExternalOutput")

    with tile.TileContext(nc) as tc:
        with tc.tile_pool(name="pool", bufs=2) as pool, \
             tc.tile_pool(name="hgp", bufs=3) as hgp, \
             tc.tile_pool(name="psum", bufs=1, space="PSUM") as psum_pool, \
             tc.tile_pool(name="const", bufs=1) as cpool, \
             nc.allow_low_precision(reason="softmax weights tolerate bf16"):

            # mask[p, g] = 1 if partition p belongs to group g
            mask_t = cpool.tile([128, N_GROUPS], dt.bfloat16)
            nc.vector.memset(mask_t[:], 0.0)
            for g in range(N_GROUPS):
                nc.vector.memset(mask_t[g * G:(g + 1) * G, g:g + 1], 1.0)

            for rep in range(reps):
              for slot in range(BPC):
                psum_uwT = psum_pool.tile([W, N_GID], dt.float32,
                                          tag=f"uwT{slot}")
                for tau in range(N_TILES):
                    cc_t = pool.tile([128, W], dt.float32, tag="cc")
                    nc.sync.dma_start(out=cc_t[:], in_=cc_d.ap()[slot, tau])
                    tsd_t = pool.tile([128, C], dt.float32, tag="tsd")
                    nc.sync.dma_start(out=tsd_t[:], in_=tsd_d.ap()[slot, tau])

                    # d[p, w, i] = cc[p, w] - tsd[p, i]
                    d_t = pool.tile([128, W, C], dt.bfloat16, tag="d")
                    nc.vector.tensor_tensor(
                        out=d_t[:],
                        in0=cc_t[:].unsqueeze(2).broadcast_to([128, W, C]),
                        in1=tsd_t[:].unsqueeze(1).broadcast_to([128, W, C]),
                        op=mybir.AluOpType.subtract)

                    sq_t = pool.tile([128, W, C], dt.bfloat16, tag="sq")
                    nc.scalar.square(out=sq_t[:], in_=d_t[:])

                    # e = exp(-sq / TEMP^2)
                    e_t = pool.tile([128, W, C], dt.bfloat16, tag="e")
                    nc.scalar.activation(out=e_t[:], in_=sq_t[:],
                                         func=mybir.ActivationFunctionType.Exp,
                                         scale=-(TEMP ** -2))

                    # softmax row sums over the window: s[p, i] = sum_w e
                    s_t = pool.tile([128, C], dt.float32, tag="s")
                    nc.vector.reduce_sum(out=s_t[:],
                                         in_=e_t[:].transpose([0, 2, 1]),
                                         axis=mybir.AxisListType.X)
                    r_t = pool.tile([128, C], dt.bfloat16, tag="r")
                    nc.vector.reciprocal(out=r_t[:], in_=s_t[:])

                    # normalized weights m = e * r
                    m_t = pool.tile([128, W, C], dt.bfloat16, tag="m")
                    nc.vector.tensor_tensor(
                        out=m_t[:], in0=e_t[:],
                        in1=r_t[:].unsqueeze(1).broadcast_to([128, W, C]),
                        op=mybir.AluOpType.mult)

                    # u[p, w] = sum_i m[p, w, i]
                    u_t = pool.tile([128, W], dt.bfloat16, tag="u")
                    nc.vector.reduce_sum(out=u_t[:], in_=m_t[:],
                                         axis=mybir.AxisListType.X)

                    # uwT[w, groups of tau] += u.T @ mask
                    nc.tensor.matmul(
                        out=psum_uwT[:, tau * N_GROUPS:(tau + 1) * N_GROUPS],
                        lhsT=u_t[:], rhs=mask_t[:],
                        start=(tau == 0), stop=(tau == N_TILES - 1))

                uwT_sb = pool.tile([W, N_GID], dt.bfloat16, tag="uwTs")
                nc.scalar.copy(out=uwT_sb[:], in_=psum_uwT[:])

                # out[d] = sum_gid uwT[:, gid].T @ hgt[gid]
                psum_out = psum_pool.tile([1, D], dt.float32, tag=f"out{slot}")
                for gid in range(N_GID):
                    hg_t = hgp.tile([W, D], dt.bfloat16, tag="hg")
                    nc.sync.dma_start(out=hg_t[:], in_=hgt_d.ap()[slot, gid])
                    nc.tensor.matmul(out=psum_out[:],
                                     lhsT=uwT_sb[:, gid:gid + 1], rhs=hg_t[:],
                                     start=(gid == 0), stop=(gid == N_GID - 1))

                out_sb = pool.tile([1, D], dt.float32, tag="osb")
                nc.scalar.copy(out=out_sb[:], in_=psum_out[:])
                nc.sync.dma_start(out=out_d.ap()[slot].unsqueeze(0),
                                  in_=out_sb[:])

    nc.compile()
    _build_cache[(W, reps)] = nc
    return nc


def _prepare(hidden_state, centers, audio_timestamps):
    """Host planning + gathers. Returns (W, in_maps)."""
    hidden_state = np.ascontiguousarray(hidden_state, dtype=np.float32)
    centers = np.ascontiguousarray(centers, dtype=np.float32)
    ts = np.ascontiguousarray(audio_timestamps, dtype=np.float32)

    sorted_ok = all(np.all(np.diff(centers[b]) >= 0) for b in range(B))

    all_starts = np.zeros((B, N_TILES, N_GROUPS), dtype=np.int64)
    W = 0
    if sorted_ok:
        for b in range(B):
            st, w = _plan_windows(centers[b], ts[b])
            all_starts[b] = st
            W = max(W, w)
        W = min((W + 7) // 8 * 8, T)
    if not sorted_ok or W >= T:
        W = T
        all_starts[:] = 0
    all_starts = np.clip(np.minimum(all_starts, T - W), 0, None)

    # gathers (vectorized): windows[b, tau, g] -> slice of centers / hidden
    idx = all_starts[..., None] + np.arange(W)          # [B, nt, ng, W]
    cc2 = np.empty((B, N_TILES, 128, W), dtype=np.float32)
    tsd = np.empty((B, N_TILES, 128, C), dtype=np.float32)
    hgt = np.empty((B, N_GID, W, D), dtype=ml_dtypes.bfloat16)
    for b in range(B):
        cwin = centers[b][idx[b]]                       # [nt, ng, W]
        # per-partition: group g of tile tau covers partitions [gG, (g+1)G)
        cc2[b] = np.repeat(cwin, G, axis=1)             # [nt, 128, W]
        tsb = ts[b].reshape(N_TILES, 128, C)
        base = tsb[:, :, 0:1]
        cc2[b] -= base
        tsd[b] = tsb - base
        # hgt[gid, w, d] = hidden[b, d, t_start+w]
        hg = hidden_state[b][:, idx[b].reshape(N_GID, W)]   # [D, ngid, W]
        hgt[b] = hg.transpose(1, 2, 0).astype(ml_dtypes.bfloat16)

    in_maps = []
    for k in range(N_CORES):
        bs = slice(k * BPC, (k + 1) * BPC)
        in_maps.append({
            "cc2": np.ascontiguousarray(cc2[bs]),
            "tsd": np.ascontiguousarray(tsd[bs]),
            "hgt": np.ascontiguousarray(hgt[bs]),
        })
    return W, in_maps


def kernel(hidden_state, centers, audio_timestamps):
    W, in_maps = _prepare(hidden_state, centers, audio_timestamps)
    nc = _build(W)
    res = run_bass_kernel_spmd(nc, in_maps, core_ids=list(range(N_CORES)))
    out = np.empty((B, D), dtype=np.float32)
    for k in range(N_CORES):
        out[k * BPC:(k + 1) * BPC] = res.results[k]["out"]
    return out
